# revision 1
# baseline (speedup 1.0000x reference)
"""Trainium2 Bass kernel for the ConstraintLoss problem (8-core SPMD).

Contract: kernel(**inputs) takes the FULL unsharded inputs (numpy or jax
arrays, keyed as in setup_inputs()) and returns the full output — the
8-tuple of scalar losses stacked into a float32 array of shape (8,):
  [L_total, L_recon, L_rule, L_attn, L_attn_gat, L_attn_rule, L_reg,
   num_violations]

Sharding strategy (host side = structure prep + shard/unshard only; all
floating-point reduction math runs on the 8 NeuronCores):
  * Cars (180000 of them) are sharded by ordinal range across the 8 cores
    (22528 rows/core, padded). model/rule/beta score vectors follow the
    same row split.
  * The edge-wise segment-max over source-node segments is turned into a
    dense per-car reduction: on the host we bin each car's rule-edge
    alphas (edges whose dst is a light/stop) into a padded [rows, K=32]
    table (payload alpha+1, 0 = empty slot).  Each core row-max-reduces
    its table shard on the vector engine — this is the distributed
    segment-max from the sharding hint, with the node space sharded so no
    cross-core max combine is needed.
  * param0/param1 are sharded by rows (512 rows of each per core); each
    core computes its partial sum of squares (param0 on the ACT engine,
    param1 on the DVE engine, overlapped with the DMA stream).
  * Each core emits 8 partial sums; the host all-reduces (adds) the 8
    partial vectors and applies the final scalar formula.
"""

import numpy as np
from contextlib import ExitStack

import concourse.bacc as bacc
import concourse.mybir as mybir
import concourse.tile as tile
from concourse import bass_isa
from concourse.bass_utils import run_bass_kernel_spmd

F32 = mybir.dt.float32
ALU = mybir.AluOpType
ACTF = mybir.ActivationFunctionType

# Problem constants (hardcoded per the task contract).
N_CAR = 180000
N = 200000
NCORES = 8

G = 176                   # row groups per partition
RPC = 128 * G             # 22528 rows (car ordinals) per core
ROWS = RPC * NCORES       # 180224 padded rows
NPAD = ROWS - N_CAR       # 224 pad rows (all on core 7)
K = 32                    # padded rule-edge slots per car
PF = 512 * 4096 // 128    # 16384 param elems per partition per core
PT = 8                    # param tiles
PTF = PF // PT            # 2048

LAMBDA_RECON, LAMBDA_RULE, LAMBDA_ATTN, LAMBDA_REG = 1.0, 0.5, 0.3, 1e-4
W_ATTN_GAT, W_ATTN_RULE = 0.5, 0.5

# BCE pad rows carry ms=0.5, rsb=0.5 -> each contributes exactly
# ln(0.5) to the bce sum independent of rsb; subtracted on the host.
_PAD_MS = 0.5

_NC = None


def _build_nc(sections=("params", "scores", "tab", "reduce"), repeat=1):
    """Build + compile the per-core Bass program (SPMD, identical on all
    cores; only the input shards differ). `sections` is for debugging —
    disabled sections leave their partial slots as memset zeros.
    `repeat` unrolls the whole body N times (timing variant: isolates
    device time from the host/RPC dispatch floor)."""
    sections = set(sections)
    nc = bacc.Bacc("TRN2", target_bir_lowering=False, debug=False,
                   enable_asserts=True, num_devices=NCORES)

    ptab = nc.dram_tensor("ptab", [128, G * K], F32, kind="ExternalInput")
    ms = nc.dram_tensor("ms", [128, G], F32, kind="ExternalInput")
    rsb = nc.dram_tensor("rsb", [128, G], F32, kind="ExternalInput")
    rsm = nc.dram_tensor("rsm", [128, G], F32, kind="ExternalInput")
    bet = nc.dram_tensor("bet", [128, G], F32, kind="ExternalInput")
    p0 = nc.dram_tensor("p0", [128, PF], F32, kind="ExternalInput")
    p1 = nc.dram_tensor("p1", [128, PF], F32, kind="ExternalInput")
    out = nc.dram_tensor("partials", [1, 8], F32, kind="ExternalOutput")

    with ExitStack() as ctx:
        tc = ctx.enter_context(tile.TileContext(nc))
        sc = ctx.enter_context(tc.tile_pool(name="scores", bufs=2))
        pp = ctx.enter_context(tc.tile_pool(name="params", bufs=3))
        scr = ctx.enter_context(tc.tile_pool(name="scratch", bufs=1))

        for _rep in range(repeat):
            # ---- L_reg partials: p0 via ACT Square+accum, p1 via DVE ttr ----
            acc0 = scr.tile([128, PT], F32)
            acc1 = scr.tile([128, PT], F32)
            sq_scr = scr.tile([128, PTF], F32)
            mul_scr = scr.tile([128, PTF], F32)
            nc.vector.memset(acc0[:], 0.0)
            nc.vector.memset(acc1[:], 0.0)
            if "params" in sections:
                for t in range(PT):
                    tp0 = pp.tile([128, PTF], F32, tag="tp0")
                    nc.gpsimd.dma_start(tp0[:], p0.ap()[:, t * PTF:(t + 1) * PTF])
                    nc.scalar.activation(sq_scr[:], tp0[:], ACTF.Square,
                                         accum_out=acc0[:, t:t + 1])
                    tp1 = pp.tile([128, PTF], F32, tag="tp1")
                    nc.sync.dma_start(tp1[:], p1.ap()[:, t * PTF:(t + 1) * PTF])
                    nc.vector.tensor_tensor(mul_scr[:], tp1[:], tp1[:], ALU.mult)
                    nc.vector.tensor_reduce(acc1[:, t:t + 1], mul_scr[:],
                                            mybir.AxisListType.X, ALU.add)

            # ---- score tiles ----
            do_scores = "scores" in sections
            do_tab = "tab" in sections
            t_ms = sc.tile([128, G], F32)
            t_rsb = sc.tile([128, G], F32)
            t_rsm = sc.tile([128, G], F32)
            t_bet = sc.tile([128, G], F32)
            if do_scores:
                nc.scalar.dma_start(t_ms[:], ms.ap())
                nc.scalar.dma_start(t_rsb[:], rsb.ap())
                nc.scalar.dma_start(t_rsm[:], rsm.ap())
                nc.scalar.dma_start(t_bet[:], bet.ap())
            t_tab = sc.tile([128, G * K], F32)
            if do_tab:
                nc.scalar.dma_start(t_tab[:], ptab.ap())

            # ---- BCE: sum rs*max(ln(ms),-100) + (1-rs)*max(ln(1-ms),-100) ----
            sbce = sc.tile([128, 1], F32)
            srule = sc.tile([128, 1], F32)
            nv = sc.tile([128, 1], F32)
            sar = sc.tile([128, 1], F32)
            scnt = sc.tile([128, 1], F32)
            sgat = sc.tile([128, 1], F32)
            for z in (sbce, srule, nv, sar, scnt, sgat):
                nc.vector.memset(z[:], 0.0)
            viol = sc.tile([128, G], F32)
            if do_scores:
             ln1 = sc.tile([128, G], F32)
             nc.scalar.activation(ln1[:], t_ms[:], ACTF.Ln)
             nc.vector.tensor_scalar_max(ln1[:], ln1[:], -100.0)
             ln2 = sc.tile([128, G], F32)
             nc.scalar.activation(ln2[:], t_ms[:], ACTF.Ln, scale=-1.0, bias=1.0)
             nc.vector.tensor_scalar_max(ln2[:], ln2[:], -100.0)
             u = sc.tile([128, G], F32)      # 1 - rsb
             nc.vector.tensor_scalar(u[:], t_rsb[:], -1.0, 1.0, ALU.mult, ALU.add)
             x1 = sc.tile([128, G], F32)
             nc.vector.tensor_tensor(x1[:], t_rsb[:], ln1[:], ALU.mult)
             x2 = sc.tile([128, G], F32)
             nc.vector.tensor_tensor(x2[:], u[:], ln2[:], ALU.mult)
             x3 = sc.tile([128, G], F32)
             nc.vector.tensor_tensor(x3[:], x1[:], x2[:], ALU.add)
             nc.vector.tensor_reduce(sbce[:], x3[:], mybir.AxisListType.X, ALU.add)
             # ---- L_rule: sum (ms - rs)^2 ----
             diff = sc.tile([128, G], F32)
             nc.vector.tensor_tensor(diff[:], t_ms[:], t_rsb[:], ALU.subtract)
             d2r = sc.tile([128, G], F32)
             nc.vector.tensor_tensor(d2r[:], diff[:], diff[:], ALU.mult)
             nc.vector.tensor_reduce(srule[:], d2r[:], mybir.AxisListType.X, ALU.add)
             # ---- violation mask + count ----
             nc.vector.tensor_scalar(viol[:], t_rsm[:], 0.5, 0.0,
                                     ALU.is_gt, ALU.add, accum_out=nv[:])
             # ---- L_attn_rule numerator: sum viol*(1-beta)^2 ----
             bsq = sc.tile([128, G], F32)
             nc.scalar.activation(bsq[:], t_bet[:], ACTF.Square, scale=-1.0, bias=1.0)
             arx = sc.tile([128, G], F32)
             nc.vector.tensor_tensor(arx[:], viol[:], bsq[:], ALU.mult)
             nc.vector.tensor_reduce(sar[:], arx[:], mybir.AxisListType.X, ALU.add)
            if do_tab and do_scores:
             # ---- GAT attn: rowmax of padded table; payload = alpha+1 ----
             rowmax = sc.tile([128, G], F32)
             nc.vector.tensor_reduce(rowmax[:],
                                     t_tab[:].rearrange("p (g k) -> p g k", k=K),
                                     mybir.AxisListType.X, ALU.max)
             has = sc.tile([128, G], F32)    # car has >=1 rule edge
             nc.vector.tensor_scalar(has[:], rowmax[:], 1.0, None, ALU.is_ge)
             dd = sc.tile([128, G], F32)     # 2 - payload == 1 - max_alpha (exact)
             nc.vector.tensor_scalar(dd[:], rowmax[:], -1.0, 2.0, ALU.mult, ALU.add)
             dd2 = sc.tile([128, G], F32)
             nc.scalar.activation(dd2[:], dd[:], ACTF.Square)
             valid = sc.tile([128, G], F32)
             nc.vector.tensor_tensor(valid[:], has[:], viol[:], ALU.mult)
             nc.vector.tensor_reduce(scnt[:], valid[:], mybir.AxisListType.X, ALU.add)
             gx = sc.tile([128, G], F32)
             nc.vector.tensor_tensor(gx[:], valid[:], dd2[:], ALU.mult)
             nc.vector.tensor_reduce(sgat[:], gx[:], mybir.AxisListType.X, ALU.add)

            # ---- collapse param accums ----
            sp0 = sc.tile([128, 1], F32)
            nc.vector.tensor_reduce(sp0[:], acc0[:], mybir.AxisListType.X, ALU.add)
            sp1 = sc.tile([128, 1], F32)
            nc.vector.tensor_reduce(sp1[:], acc1[:], mybir.AxisListType.X, ALU.add)

            # ---- gather 8 partials, cross-partition add, DMA out ----
            parts = sc.tile([128, 8], F32)
            for j, a in enumerate([sbce, srule, nv, sar, scnt, sgat, sp0, sp1]):
                nc.vector.tensor_copy(parts[:, j:j + 1], a[:])
            red = sc.tile([128, 8], F32)
            if "reduce" in sections:
                nc.gpsimd.partition_all_reduce(red[:], parts[:], channels=128,
                                               reduce_op=bass_isa.ReduceOp.add)
            else:
                nc.vector.tensor_copy(red[:], parts[:])
            nc.gpsimd.dma_start(out.ap(), red[0:1, :])

    nc.compile()
    return nc


def _get_nc():
    global _NC
    if _NC is None:
        _NC = _build_nc()
    return _NC


def prep_in_maps(inputs):
    """Host-side structure prep + sharding. Returns per-core input dicts."""
    ms = np.asarray(inputs["model_scores"], np.float32)
    rs = np.asarray(inputs["rule_scores"], np.float32)
    alpha = np.asarray(inputs["alpha_gat"], np.float32)
    beta = np.asarray(inputs["beta_rule"], np.float32)
    ei = np.asarray(inputs["edge_index"])
    et = np.asarray(inputs["entity_types"])
    p0 = np.ascontiguousarray(np.asarray(inputs["param0"], np.float32))
    p1 = np.ascontiguousarray(np.asarray(inputs["param1"], np.float32))

    src = ei[0].astype(np.int64, copy=False)
    dst = ei[1].astype(np.int64, copy=False)

    # rule edges: dst is a light (1) or stop line (2)
    rule_node = (et == 1) | (et == 2)
    sel = rule_node[dst]
    src_r = src[sel]
    a_r = alpha[sel]

    # group rule-edge alphas by source node (CSR-style)
    order = np.argsort(src_r, kind="stable")
    ssrc = src_r[order]
    sa = a_r[order]
    counts = np.bincount(ssrc, minlength=N)
    starts = np.zeros_like(counts)
    starts[1:] = np.cumsum(counts[:-1])

    # car ordinal -> node id (reference: nonzero(et==0, size=N_CAR), fill 0)
    car_ids = np.nonzero(et == 0)[0]
    if car_ids.size >= N_CAR:
        car_ids = car_ids[:N_CAR]
    else:
        car_ids = np.concatenate(
            [car_ids, np.zeros(N_CAR - car_ids.size, car_ids.dtype)])

    # padded [ROWS, K] table of (alpha+1), one row per car ordinal
    cnt_full = counts[car_ids]
    cnt_ord = np.minimum(cnt_full, K)
    tot = int(cnt_ord.sum())
    row_idx = np.repeat(np.arange(N_CAR, dtype=np.int64), cnt_ord)
    cum = np.cumsum(cnt_ord) - cnt_ord
    within = np.arange(tot, dtype=np.int64) - np.repeat(cum, cnt_ord)
    srcpos = np.repeat(starts[car_ids], cnt_ord) + within
    ptab = np.zeros(ROWS * K, np.float32)
    ptab[row_idx * K + within] = sa[srcpos] + np.float32(1.0)
    # overflow fold (degree > K; not hit for the graded distribution)
    for i in np.nonzero(cnt_full > K)[0]:
        node = car_ids[i]
        extra = sa[starts[node] + K:starts[node] + cnt_full[i]] + np.float32(1.0)
        ptab[i * K + K - 1] = max(ptab[i * K + K - 1], extra.max())
    ptab = ptab.reshape(ROWS, K)

    # padded score rows
    def pad(v, fill):
        o = np.full(ROWS, fill, np.float32)
        o[:N_CAR] = v
        return o

    ms_p = pad(ms, _PAD_MS)
    rsb_p = pad(rs, _PAD_MS)   # pad makes bce term exactly ln(0.5), rule term 0
    rsm_p = pad(rs, 0.0)       # pad never counts as violation
    bet_p = pad(beta, 1.0)

    in_maps = []
    for c in range(NCORES):
        r0, r1 = c * RPC, (c + 1) * RPC
        in_maps.append({
            "ptab": np.ascontiguousarray(ptab[r0:r1]).reshape(128, G * K),
            "ms": ms_p[r0:r1].reshape(128, G),
            "rsb": rsb_p[r0:r1].reshape(128, G),
            "rsm": rsm_p[r0:r1].reshape(128, G),
            "bet": bet_p[r0:r1].reshape(128, G),
            "p0": p0[c * 512:(c + 1) * 512].reshape(128, PF),
            "p1": p1[c * 512:(c + 1) * 512].reshape(128, PF),
        })
    return in_maps


def combine_partials(partials_per_core):
    """Host unshard: add the 8 partial vectors, apply the scalar formula."""
    s = np.zeros(8, np.float64)
    for p in partials_per_core:
        s += np.asarray(p, np.float64).reshape(-1)[:8]
    s_bce, s_rule, nv, s_ar, s_cnt, s_gat, s_p0, s_p1 = s
    s_bce -= NPAD * np.log(0.5)  # remove the constant pad-row contribution

    L_recon = -s_bce / N_CAR
    L_rule = s_rule / N_CAR
    any_viol = nv > 0
    L_attn_gat = (s_gat / max(s_cnt, 1.0)) if (any_viol and s_cnt > 0) else 0.0
    L_attn_rule = (s_ar / max(nv, 1.0)) if any_viol else 0.0
    L_attn = W_ATTN_GAT * L_attn_gat + W_ATTN_RULE * L_attn_rule
    L_reg = s_p0 + s_p1
    L_total = (LAMBDA_RECON * L_recon + LAMBDA_RULE * L_rule
               + LAMBDA_ATTN * L_attn + LAMBDA_REG * L_reg)
    return np.array([L_total, L_recon, L_rule, L_attn, L_attn_gat,
                     L_attn_rule, L_reg, nv], np.float32)


def kernel(**inputs):
    nc = _get_nc()
    in_maps = prep_in_maps(inputs)
    res = run_bass_kernel_spmd(nc, in_maps, list(range(NCORES)))
    return combine_partials([r["partials"] for r in res.results])

